# revision 7
# baseline (speedup 1.0000x reference)
"""Trainium2 Bass kernel for MemoryEfficientAttention.

Model: out = softmax((x@Wq)(x@Wk)^T / sqrt(dk)) (x@Wv) @ W_out
  x [2, 4096, 512], W_qkv [512, 1536], W_out [512, 512], H=8, dk=64.

Distribution across 8 NeuronCores (no collectives):
  device d handles batch b = d//4 and query rows [(d%4)*1024, +1024).
  Each device computes k/v projections for its full batch (4x redundant),
  q projection for its row slice, attention in the transposed (S^T)
  orientation so the P@v matmul needs no transposes, and its slice of the
  output projection, emitted transposed [512, 1024]; the host stitches the
  full [2, 4096, 512] output back together.

Numerics: matmuls run as float32r (TF32-like, ~1.5e-4 rel err, full PE
speed at N>=256); softmax skips the max-subtraction (scores are O(1) by
construction) and folds the 1/sqrt(dk) scale into the Exp activation; the
softmax denominator comes from a ones-column appended to v.
"""

import sys

for _p in ("/opt/trn_rl_repo",):
    if _p not in sys.path:
        sys.path.insert(0, _p)

import json

import numpy as np

import concourse.bass as bass
import concourse.bass2jax as _b2j
import concourse.bass_utils as _bu
import concourse.mybir as mybir
import concourse.tile as tile

# ---------------------------------------------------------------------------
# Workaround: this walrus build rejects >1 sync wait per instruction. Split
# excess on_wait entries onto injected single-wait EventSemaphore
# instructions on the same engine right before the original instruction.
# ---------------------------------------------------------------------------
_orig_compile_bir_kernel = _bu.compile_bir_kernel


def _split_excess_waits(bir_bytes):
    bir = json.loads(bir_bytes)
    n = 0
    for fn in bir.get("functions", []):
        for blk in fn.get("blocks", []):
            out = []
            for ins in blk.get("instructions", []):
                si = ins.get("sync_info")
                if si:
                    ow = si.get("on_wait") or []
                    if len(ow) > 1:
                        for w in ow[:-1]:
                            n += 1
                            out.append({
                                "debug": ins.get("debug", 0),
                                "engine": ins["engine"],
                                "ins": [],
                                "outs": [],
                                "name": f"{ins['name']}-xw{n}",
                                "opcode": "EventSemaphore",
                                "sync_info": {"on_update": [], "on_wait": [w]},
                            })
                        si["on_wait"] = [ow[-1]]
                out.append(ins)
            blk["instructions"] = out
    return json.dumps(bir).encode()


def _patched_compile_bir_kernel(bir_json, tmpdir, neff_name="file.neff"):
    if isinstance(bir_json, str):
        bir_json = bir_json.encode()
    return _orig_compile_bir_kernel(_split_excess_waits(bir_json), tmpdir, neff_name)


if getattr(_bu.compile_bir_kernel, "__name__", "") != "_patched_compile_bir_kernel":
    _bu.compile_bir_kernel = _patched_compile_bir_kernel
    _b2j.compile_bir_kernel = _patched_compile_bir_kernel

# ---------------------------------------------------------------------------
# Problem constants (hardcoded per the harness contract)
# ---------------------------------------------------------------------------
B, S, D = 2, 4096, 512
H, DK = 8, 64
NDEV = 8
ROWS = (B * S) // NDEV          # 1024 query rows per device
DEV_PER_BATCH = NDEV // B       # 4
NT = S // 128                   # 32 key tiles
NTB = S // 512                  # 8 projection t-blocks
NPAIR = H // 2                  # 4 head pairs
VW = 65                         # v width per head incl. ones column

f32 = mybir.dt.float32
f32r = mybir.dt.float32r
EXPF = mybir.ActivationFunctionType.Exp


def _build_nc():
    nc = bass.Bass()
    xT = nc.dram_tensor("xT", [D, S], f32, kind="ExternalInput")
    wqkv = nc.dram_tensor("wqkv", [D, 3 * D], f32, kind="ExternalInput")
    wout = nc.dram_tensor("wout", [D, D], f32, kind="ExternalInput")
    xTq = nc.dram_tensor("xTq", [D, ROWS], f32, kind="ExternalInput")
    outT = nc.dram_tensor("outT", [D, ROWS], f32, kind="ExternalOutput")

    with tile.TileContext(nc) as tc:
        with tc.tile_pool(name="kvq", bufs=1) as kvq, \
             tc.tile_pool(name="np_", bufs=1) as np_, \
             tc.tile_pool(name="dram", bufs=1, space="DRAM") as dramp:

            # persistent activations
            kT = [kvq.tile([128, S], f32r, tag=f"kT{c}", name=f"kT{c}")
                  for c in range(4)]
            qT = [kvq.tile([128, ROWS], f32r, tag=f"qT{c}", name=f"qT{c}")
                  for c in range(4)]
            vsb = kvq.tile([128, NT * H * VW], f32r, tag="vsb", name="vsb")
            nctx = [np_.tile([128, ROWS], f32r, tag=f"nctx{p}", name=f"nctx{p}")
                    for p in range(NPAIR)]
            dz = dramp.tile([1, H * ROWS], f32, name="dz")

            # ones columns of v: fill a contiguous tile, strided-copy into slots
            ones8 = np_.tile([128, NT * H], f32, tag="ones8", name="ones8")
            nc.gpsimd.memset(ones8[:], 1.0)
            nc.vector.tensor_copy(
                vsb[:].rearrange("p (c w) -> p c w", w=VW)[:, :, DK:DK + 1],
                ones8[:].rearrange("p (c f) -> p c f", f=1))

            # ================= Phase 1: projections =================
            with tc.tile_pool(name="wq", bufs=1) as wq, \
                 tc.tile_pool(name="xs", bufs=2) as xs, \
                 tc.tile_pool(name="ps_a", bufs=2, space="PSUM") as ps_a:
                wqt = wq.tile([128, 4 * 3 * D], f32r, name="wqt")  # 4 d-chunks
                nc.sync.dma_start(
                    wqt[:].rearrange("p (c f) -> p c f", c=4),
                    wqkv[:, :].rearrange("(c p) f -> p c f", p=128).bitcast(f32r))

                def wslice(i, lo, hi):
                    return wqt[:, i * 3 * D + lo:i * 3 * D + hi]

                for tb in range(NTB):
                    xc = xs.tile([128, 4 * 512], f32r, tag="xc", name="xc")
                    nc.sync.dma_start(
                        xc[:].rearrange("p (c f) -> p c f", c=4),
                        xT[:, tb * 512:(tb + 1) * 512]
                        .rearrange("(c p) f -> p c f", p=128).bitcast(f32r))
                    # kT (all heads): lhsT = W_k chunk, rhs = xT chunk
                    for c in range(4):
                        pk = ps_a.tile([128, 512], f32, tag="pk", name="pk")
                        for i in range(4):
                            nc.tensor.matmul(
                                pk[:],
                                wslice(i, D + c * 128, D + (c + 1) * 128),
                                xc[:, i * 512:(i + 1) * 512],
                                start=(i == 0), stop=(i == 3))
                        nc.vector.tensor_copy(
                            kT[c][:, tb * 512:(tb + 1) * 512], pk[:])
                    # v (all heads, natural [t, e], VW-stride layout)
                    for tt in range(4):
                        gt = tb * 4 + tt
                        pv = ps_a.tile([128, 512], f32, tag="pv", name="pv")
                        for i in range(4):
                            nc.tensor.matmul(
                                pv[:],
                                xc[:, i * 512 + tt * 128:i * 512 + (tt + 1) * 128],
                                wslice(i, 2 * D, 3 * D),
                                start=(i == 0), stop=(i == 3))
                        vdst = vsb[:, gt * H * VW:(gt + 1) * H * VW].rearrange(
                            "p (h w) -> p h w", w=VW)[:, :, 0:DK]
                        vsrc = pv[:].rearrange("p (h e) -> p h e", e=DK)
                        nc.vector.tensor_copy(vdst, vsrc)

                # q projection for this device's row slice
                for seg in range(2):
                    xq = xs.tile([128, 4 * 512], f32r, tag="xc", name="xq")
                    nc.sync.dma_start(
                        xq[:].rearrange("p (c f) -> p c f", c=4),
                        xTq[:, seg * 512:(seg + 1) * 512]
                        .rearrange("(c p) f -> p c f", p=128).bitcast(f32r))
                    for c in range(4):
                        pq = ps_a.tile([128, 512], f32, tag="pk", name="pq")
                        for i in range(4):
                            nc.tensor.matmul(
                                pq[:],
                                wslice(i, c * 128, (c + 1) * 128),
                                xq[:, i * 512:(i + 1) * 512],
                                start=(i == 0), stop=(i == 3))
                        nc.vector.tensor_copy(
                            qT[c][:, seg * 512:(seg + 1) * 512], pq[:])

            # ================= Phase 2: attention =================
            with tc.tile_pool(name="ptp", bufs=2) as ptp, \
                 tc.tile_pool(name="zp", bufs=1) as zp, \
                 tc.tile_pool(name="ps_s", bufs=1, space="PSUM") as ps_s, \
                 tc.tile_pool(name="ps_c", bufs=1, space="PSUM") as ps_c:
                for p in range(NPAIR):
                    c0 = ps_c.tile([VW, ROWS], f32, tag="c0", name="c0")
                    c1 = ps_c.tile([VW, ROWS], f32, tag="c1", name="c1")
                    for t in range(NT):
                        st = ps_s.tile([128, 2048], f32, tag="st", name="st")
                        for seg in range(2):
                            nc.tensor.matmul(
                                st[:, seg * 512:(seg + 1) * 512],
                                kT[p][0:64, t * 128:(t + 1) * 128],
                                qT[p][0:64, seg * 512:(seg + 1) * 512],
                                start=True, stop=True)
                            nc.tensor.matmul(
                                st[:, 1024 + seg * 512:1024 + (seg + 1) * 512],
                                kT[p][64:128, t * 128:(t + 1) * 128],
                                qT[p][64:128, seg * 512:(seg + 1) * 512],
                                start=True, stop=True)
                        pt = ptp.tile([128, 2048], f32r, tag="pt", name="pt")
                        nc.scalar.activation(pt[:], st[:], EXPF, scale=0.125)
                        for seg in range(2):
                            nc.tensor.matmul(
                                c0[:, seg * 512:(seg + 1) * 512],
                                vsb[:, (t * H + 2 * p) * VW:(t * H + 2 * p) * VW + VW],
                                pt[:, seg * 512:(seg + 1) * 512],
                                start=(t == 0), stop=(t == NT - 1))
                            nc.tensor.matmul(
                                c1[:, seg * 512:(seg + 1) * 512],
                                vsb[:, (t * H + 2 * p + 1) * VW:
                                     (t * H + 2 * p + 1) * VW + VW],
                                pt[:, 1024 + seg * 512:1024 + (seg + 1) * 512],
                                start=(t == 0), stop=(t == NT - 1))
                    # normalize: 1/Z, bounced via DRAM to broadcast over rows
                    zt = zp.tile([64, ROWS], f32, tag="zt", name="zt")
                    nc.vector.tensor_copy(zt[0:1, :], c0[64:65, :])
                    nc.vector.tensor_copy(zt[32:33, :], c1[64:65, :])
                    rzt = zp.tile([64, ROWS], f32, tag="rzt", name="rzt")
                    nc.vector.reciprocal(rzt[:], zt[:])
                    nc.sync.dma_start(
                        dz[0:1, 2 * p * ROWS:(2 * p + 1) * ROWS], rzt[0:1, :])
                    nc.sync.dma_start(
                        dz[0:1, (2 * p + 1) * ROWS:(2 * p + 2) * ROWS], rzt[32:33, :])
                    rzb = zp.tile([128, ROWS], f32, tag="rzb", name="rzb")
                    nc.sync.dma_start(
                        rzb[0:64, :],
                        dz[0:1, 2 * p * ROWS:(2 * p + 1) * ROWS]
                        .partition_broadcast(64))
                    nc.sync.dma_start(
                        rzb[64:128, :],
                        dz[0:1, (2 * p + 1) * ROWS:(2 * p + 2) * ROWS]
                        .partition_broadcast(64))
                    nc.vector.tensor_mul(nctx[p][0:64, :], c0[0:64, :], rzb[0:64, :])
                    nc.vector.tensor_mul(nctx[p][64:128, :], c1[0:64, :],
                                         rzb[64:128, :])

            # ================= Phase 3: output projection =================
            with tc.tile_pool(name="wop", bufs=1) as wop, \
                 tc.tile_pool(name="osp", bufs=2) as osp, \
                 tc.tile_pool(name="ps_o", bufs=2, space="PSUM") as ps_o:
                wot = wop.tile([128, 4 * D], f32r, name="wot")
                nc.sync.dma_start(
                    wot[:].rearrange("p (c f) -> p c f", c=4),
                    wout[:, :].rearrange("(c p) f -> p c f", p=128).bitcast(f32r))
                for ft in range(4):
                    osb = osp.tile([128, ROWS], f32, tag="osb", name="osb")
                    for seg in range(2):
                        po = ps_o.tile([128, 512], f32, tag="po", name="po")
                        for c in range(4):
                            nc.tensor.matmul(
                                po[:],
                                wot[:, c * D + ft * 128:c * D + (ft + 1) * 128],
                                nctx[c][:, seg * 512:(seg + 1) * 512],
                                start=(c == 0), stop=(c == 3))
                        nc.vector.tensor_copy(
                            osb[:, seg * 512:(seg + 1) * 512], po[:])
                    nc.sync.dma_start(outT[ft * 128:(ft + 1) * 128, :], osb[:])
    return nc


_NC_CACHE = None


def kernel(x, W_qkv, W_out):
    global _NC_CACHE
    from concourse.bass_utils import run_bass_kernel_spmd

    x = np.asarray(x, dtype=np.float32)
    W_qkv = np.ascontiguousarray(np.asarray(W_qkv, dtype=np.float32))
    W_out = np.ascontiguousarray(np.asarray(W_out, dtype=np.float32))

    if _NC_CACHE is None:
        _NC_CACHE = _build_nc()
    nc = _NC_CACHE

    xTb = [np.ascontiguousarray(x[b].T) for b in range(B)]
    in_maps = []
    for d in range(NDEV):
        b = d // DEV_PER_BATCH
        r0 = (d % DEV_PER_BATCH) * ROWS
        in_maps.append({
            "xT": xTb[b],
            "xTq": np.ascontiguousarray(xTb[b][:, r0:r0 + ROWS]),
            "wqkv": W_qkv,
            "wout": W_out,
        })

    res = run_bass_kernel_spmd(nc, in_maps, core_ids=list(range(NDEV)))

    out = np.empty((B, S, D), dtype=np.float32)
    for d in range(NDEV):
        b = d // DEV_PER_BATCH
        r0 = (d % DEV_PER_BATCH) * ROWS
        out[b, r0:r0 + ROWS, :] = res.results[d]["outT"].T
    return out


# revision 12
# speedup vs baseline: 1.0378x; 1.0378x over previous
"""Trainium2 Bass kernel for MemoryEfficientAttention.

Model: out = softmax((x@Wq)(x@Wk)^T / sqrt(dk)) (x@Wv) @ W_out
  x [2, 4096, 512], W_qkv [512, 1536], W_out [512, 512], H=8, dk=64.

Distribution across 8 NeuronCores (no collectives):
  device d handles batch b = d//4 and query rows [(d%4)*1024, +1024).
  Each device computes k/v projections for its full batch (4x redundant),
  q projection for its row slice, attention in the transposed (S^T)
  orientation so the P@v matmul needs no transposes, and its slice of the
  output projection, emitted transposed [512, 1024]; the host stitches the
  full [2, 4096, 512] output back together.

Numerics: matmuls run as float32r (TF32-like, ~1.5e-4 rel err, full PE
speed at N>=256); softmax skips the max-subtraction (scores are O(1) by
construction) and folds the 1/sqrt(dk) scale into the Exp activation; the
softmax denominator comes from a ones-column appended to v.
"""

import sys

for _p in ("/opt/trn_rl_repo",):
    if _p not in sys.path:
        sys.path.insert(0, _p)

import json
from contextlib import ExitStack

import numpy as np

import concourse.bass as bass
import concourse.bass2jax as _b2j
import concourse.bass_utils as _bu
import concourse.mybir as mybir
import concourse.tile as tile

# ---------------------------------------------------------------------------
# Workaround: this walrus build rejects >1 sync wait per instruction. Split
# excess on_wait entries onto injected single-wait EventSemaphore
# instructions on the same engine right before the original instruction.
# ---------------------------------------------------------------------------
_orig_compile_bir_kernel = _bu.compile_bir_kernel


def _split_excess_waits(bir_bytes):
    bir = json.loads(bir_bytes)
    n = 0
    for fn in bir.get("functions", []):
        for blk in fn.get("blocks", []):
            out = []
            for ins in blk.get("instructions", []):
                si = ins.get("sync_info")
                if si:
                    ow = si.get("on_wait") or []
                    if len(ow) > 1:
                        for w in ow[:-1]:
                            n += 1
                            out.append({
                                "debug": ins.get("debug", 0),
                                "engine": ins["engine"],
                                "ins": [],
                                "outs": [],
                                "name": f"{ins['name']}-xw{n}",
                                "opcode": "EventSemaphore",
                                "sync_info": {"on_update": [], "on_wait": [w]},
                            })
                        si["on_wait"] = [ow[-1]]
                out.append(ins)
            blk["instructions"] = out
    return json.dumps(bir).encode()


def _patched_compile_bir_kernel(bir_json, tmpdir, neff_name="file.neff"):
    if isinstance(bir_json, str):
        bir_json = bir_json.encode()
    return _orig_compile_bir_kernel(_split_excess_waits(bir_json), tmpdir, neff_name)


if getattr(_bu.compile_bir_kernel, "__name__", "") != "_patched_compile_bir_kernel":
    _bu.compile_bir_kernel = _patched_compile_bir_kernel
    _b2j.compile_bir_kernel = _patched_compile_bir_kernel

# ---------------------------------------------------------------------------
# Problem constants (hardcoded per the harness contract)
# ---------------------------------------------------------------------------
B, S, D = 2, 4096, 512
H, DK = 8, 64
NDEV = 8
ROWS = (B * S) // NDEV          # 1024 query rows per device
DEV_PER_BATCH = NDEV // B       # 4
NT = S // 128                   # 32 key tiles
NTB = S // 512                  # 8 projection t-blocks
NPAIR = H // 2                  # 4 head pairs
VW = 65                         # v width per head incl. ones column

f32 = mybir.dt.float32
f32r = mybir.dt.float32r
EXPF = mybir.ActivationFunctionType.Exp


def _build_nc():
    nc = bass.Bass()
    xT = nc.dram_tensor("xT", [D, S], f32, kind="ExternalInput")
    wqkv = nc.dram_tensor("wqkv", [D, 3 * D], f32, kind="ExternalInput")
    wout = nc.dram_tensor("wout", [D, D], f32, kind="ExternalInput")
    xTq = nc.dram_tensor("xTq", [D, ROWS], f32, kind="ExternalInput")
    outT = nc.dram_tensor("outT", [D, ROWS], f32, kind="ExternalOutput")

    with tile.TileContext(nc) as tc:
        with tc.tile_pool(name="kvq", bufs=1) as kvq, \
             tc.tile_pool(name="np_", bufs=1) as np_, \
             tc.tile_pool(name="dram", bufs=1, space="DRAM") as dramp, \
             tc.tile_pool(name="ps_k", bufs=2, space="PSUM") as ps_k:

            # persistent activations
            kT = [kvq.tile([128, S], f32r, tag=f"kT{c}", name=f"kT{c}")
                  for c in range(4)]
            qT = [kvq.tile([128, ROWS], f32r, tag=f"qT{c}", name=f"qT{c}")
                  for c in range(4)]
            vsb = kvq.tile([128, NT * H * VW], f32r, tag="vsb", name="vsb")
            uctx = [np_.tile([128, ROWS], f32r, tag=f"uctx{p}", name=f"uctx{p}")
                    for p in range(NPAIR)]
            wk = np_.tile([128, 4 * 512], f32r, tag="wk", name="wk")
            dz = dramp.tile([1, H * ROWS], f32, name="dz")

            nc.sync.dma_start(
                wk[:].rearrange("p (c f) -> p c f", c=4),
                wqkv[:, D:2 * D].rearrange("(c p) f -> p c f", p=128)
                .bitcast(f32r))

            attn_scope = ExitStack()
            xs = attn_scope.enter_context(tc.tile_pool(name="xs", bufs=2))
            ptp = attn_scope.enter_context(tc.tile_pool(name="ptp", bufs=2))
            zp = attn_scope.enter_context(tc.tile_pool(name="zp", bufs=1))

            # ones columns of v: stage 1.0s in rzb (reused later), strided-copy
            ones8 = zp.tile([128, 512], f32, tag="rzb", name="ones8")
            nc.gpsimd.memset(ones8[:], 1.0)
            nc.vector.tensor_copy(
                vsb[:].rearrange("p (c w) -> p c w", w=VW)[:, :, DK:DK + 1],
                ones8[:, 0:NT * H].rearrange("p (c f) -> p c f", f=1))

            def load_x(tb, nm):
                t = xs.tile([128, 4 * 512], f32r, tag="xc", name=nm)
                nc.sync.dma_start(
                    t[:].rearrange("p (c f) -> p c f", c=4),
                    xT[:, tb * 512:(tb + 1) * 512]
                    .rearrange("(c p) f -> p c f", p=128).bitcast(f32r))
                return t

            # ---------- pass A: v (all heads) and q ----------
            with tc.tile_pool(name="wv", bufs=1) as wvp, \
                 tc.tile_pool(name="ps_v", bufs=2, space="PSUM") as ps_v:
                wv = wvp.tile([128, 4 * 512], f32r, name="wv")
                nc.sync.dma_start(
                    wv[:].rearrange("p (c f) -> p c f", c=4),
                    wqkv[:, 2 * D:3 * D].rearrange("(c p) f -> p c f", p=128)
                    .bitcast(f32r))
                for tb in range(NTB):
                    xc = load_x(tb, "xcv")
                    for tt in range(4):
                        gt = tb * 4 + tt
                        pv = ps_v.tile([128, 512], f32, tag="pv", name="pv")
                        for i in range(4):
                            nc.tensor.matmul(
                                pv[:],
                                xc[:, i * 512 + tt * 128:i * 512 + (tt + 1) * 128],
                                wv[:, i * 512:(i + 1) * 512],
                                start=(i == 0), stop=(i == 3))
                        vdst = vsb[:, gt * H * VW:(gt + 1) * H * VW].rearrange(
                            "p (h w) -> p h w", w=VW)[:, :, 0:DK]
                        nc.vector.tensor_copy(
                            vdst, pv[:].rearrange("p (h e) -> p h e", e=DK))

            with tc.tile_pool(name="wqp", bufs=1) as wqp:
                wqq = wqp.tile([128, 4 * 512], f32r, name="wqq")
                nc.sync.dma_start(
                    wqq[:].rearrange("p (c f) -> p c f", c=4),
                    wqkv[:, 0:D].rearrange("(c p) f -> p c f", p=128)
                    .bitcast(f32r))
                for seg in range(2):
                    xq = xs.tile([128, 4 * 512], f32r, tag="xc", name="xq")
                    nc.sync.dma_start(
                        xq[:].rearrange("p (c f) -> p c f", c=4),
                        xTq[:, seg * 512:(seg + 1) * 512]
                        .rearrange("(c p) f -> p c f", p=128).bitcast(f32r))
                    for c in range(4):
                        pq = ps_k.tile([128, 512], f32, tag="pk", name="pq")
                        for i in range(4):
                            nc.tensor.matmul(
                                pq[:],
                                wqq[:, i * 512 + c * 128:i * 512 + (c + 1) * 128],
                                xq[:, i * 512:(i + 1) * 512],
                                start=(i == 0), stop=(i == 3))
                        nc.vector.tensor_copy(
                            qT[c][:, seg * 512:(seg + 1) * 512], pq[:])

            # ---------- per pair: kT[p] projection, then attention ----------
            with tc.tile_pool(name="ps_s", bufs=2, space="PSUM") as ps_s, \
                 tc.tile_pool(name="ps_c", bufs=1, space="PSUM") as ps_c:
                for p in range(NPAIR):
                    for tb in range(NTB):
                        xk = load_x(tb, "xk")
                        pk = ps_k.tile([128, 512], f32, tag="pk", name="pk")
                        for i in range(4):
                            nc.tensor.matmul(
                                pk[:],
                                wk[:, i * 512 + p * 128:i * 512 + (p + 1) * 128],
                                xk[:, i * 512:(i + 1) * 512],
                                start=(i == 0), stop=(i == 3))
                        nc.vector.tensor_copy(
                            kT[p][:, tb * 512:(tb + 1) * 512], pk[:])

                    for sb in range(2):
                        s0 = sb * 512
                        c0 = ps_c.tile([VW, 512], f32, tag="c0", name="c0")
                        c1 = ps_c.tile([VW, 512], f32, tag="c1", name="c1")
                        for t in range(NT):
                            st = ps_s.tile([128, 1024], f32, tag="st", name="st")
                            nc.tensor.matmul(
                                st[:, 0:512],
                                kT[p][0:64, t * 128:(t + 1) * 128],
                                qT[p][0:64, s0:s0 + 512],
                                start=True, stop=True)
                            nc.tensor.matmul(
                                st[:, 512:1024],
                                kT[p][64:128, t * 128:(t + 1) * 128],
                                qT[p][64:128, s0:s0 + 512],
                                start=True, stop=True)
                            pt = ptp.tile([128, 1024], f32r, tag="pt", name="pt")
                            nc.scalar.activation(pt[:], st[:], EXPF, scale=0.125)
                            nc.tensor.matmul(
                                c0[:],
                                vsb[:, (t * H + 2 * p) * VW:
                                     (t * H + 2 * p) * VW + VW],
                                pt[:, 0:512],
                                start=(t == 0), stop=(t == NT - 1))
                            nc.tensor.matmul(
                                c1[:],
                                vsb[:, (t * H + 2 * p + 1) * VW:
                                     (t * H + 2 * p + 1) * VW + VW],
                                pt[:, 512:1024],
                                start=(t == 0), stop=(t == NT - 1))
                        # fast PSUM release: copy unnormalized ctx + Z to SBUF
                        zr = zp.tile([64, 1024], f32, tag="zr", name="zr")
                        nc.vector.tensor_copy(zr[0:1, 0:512], c0[64:65, :])
                        nc.vector.tensor_copy(zr[32:33, 0:512], c1[64:65, :])
                        nc.vector.tensor_copy(
                            uctx[p][0:64, s0:s0 + 512], c0[0:64, :])
                        nc.vector.tensor_copy(
                            uctx[p][64:128, s0:s0 + 512], c1[0:64, :])
                        # normalize off the critical path (DVE + DMA only)
                        nc.vector.reciprocal(zr[:, 512:1024], zr[:, 0:512])
                        o0 = 2 * p * ROWS + s0
                        o1 = (2 * p + 1) * ROWS + s0
                        nc.sync.dma_start(dz[0:1, o0:o0 + 512], zr[0:1, 512:1024])
                        nc.sync.dma_start(dz[0:1, o1:o1 + 512],
                                          zr[32:33, 512:1024])
                        rzb = zp.tile([128, 512], f32, tag="rzb", name="rzb")
                        nc.sync.dma_start(
                            rzb[0:64, :],
                            dz[0:1, o0:o0 + 512].partition_broadcast(64))
                        nc.sync.dma_start(
                            rzb[64:128, :],
                            dz[0:1, o1:o1 + 512].partition_broadcast(64))
                        nc.vector.tensor_mul(
                            uctx[p][:, s0:s0 + 512], uctx[p][:, s0:s0 + 512],
                            rzb[:])

            attn_scope.close()

            # ---------- output projection ----------
            with tc.tile_pool(name="wop", bufs=1) as wop, \
                 tc.tile_pool(name="osp", bufs=2) as osp, \
                 tc.tile_pool(name="ps_o", bufs=2, space="PSUM") as ps_o:
                wot = wop.tile([128, 4 * D], f32r, name="wot")
                nc.sync.dma_start(
                    wot[:].rearrange("p (c f) -> p c f", c=4),
                    wout[:, :].rearrange("(c p) f -> p c f", p=128).bitcast(f32r))
                for ft in range(4):
                    osb = osp.tile([128, ROWS], f32, tag="osb", name="osb")
                    for seg in range(2):
                        po = ps_o.tile([128, 512], f32, tag="po", name="po")
                        for c in range(4):
                            nc.tensor.matmul(
                                po[:],
                                wot[:, c * D + ft * 128:c * D + (ft + 1) * 128],
                                uctx[c][:, seg * 512:(seg + 1) * 512],
                                start=(c == 0), stop=(c == 3))
                        nc.vector.tensor_copy(
                            osb[:, seg * 512:(seg + 1) * 512], po[:])
                    nc.sync.dma_start(outT[ft * 128:(ft + 1) * 128, :], osb[:])
    return nc


_NC_CACHE = None


def kernel(x, W_qkv, W_out):
    global _NC_CACHE
    from concourse.bass_utils import run_bass_kernel_spmd

    x = np.asarray(x, dtype=np.float32)
    W_qkv = np.ascontiguousarray(np.asarray(W_qkv, dtype=np.float32))
    W_out = np.ascontiguousarray(np.asarray(W_out, dtype=np.float32))

    if _NC_CACHE is None:
        _NC_CACHE = _build_nc()
    nc = _NC_CACHE

    xTb = [np.ascontiguousarray(x[b].T) for b in range(B)]
    in_maps = []
    for d in range(NDEV):
        b = d // DEV_PER_BATCH
        r0 = (d % DEV_PER_BATCH) * ROWS
        in_maps.append({
            "xT": xTb[b],
            "xTq": np.ascontiguousarray(xTb[b][:, r0:r0 + ROWS]),
            "wqkv": W_qkv,
            "wout": W_out,
        })

    res = run_bass_kernel_spmd(nc, in_maps, core_ids=list(range(NDEV)))

    out = np.empty((B, S, D), dtype=np.float32)
    for d in range(NDEV):
        b = d // DEV_PER_BATCH
        r0 = (d % DEV_PER_BATCH) * ROWS
        out[b, r0:r0 + ROWS, :] = res.results[d]["outT"].T
    return out
